# revision 18
# baseline (speedup 1.0000x reference)
"""AgentAwareAttention Trainium2 kernel (8 NeuronCores, SPMD).

Sharding: core c -> batch b=c//4, query-row block r0 = 510*(c%4).
Keys are ROTATED per core by r0 so every SBUF offset is core-independent
(pure SPMD).  Host unshard = np.roll (inverse rotation) + concat.

Per core (all 8 heads, query rows [r0, r0+510), keys all 2009 rotated):
  phase 0: load xT (pre-transposed on host), project kT/ksT/qT/qsT (d-major)
           and v (j-major, ones-augmented 65th column for row sums).
  per head:
    pass B ([key, query] layout): dotsT = k^T q tiles -> blend block-diag
           self scores (copy_predicated w/ host mask) -> exp -> accumulate
           outT[65,510] = v_aug^T @ E^T  (row 64 = softmax denominators)
           -> outT scaled by 1/sums (broadcast DMA) -> SBUF per-head oT.
  pass A ([query, key] layout): dots tiles -> blend -> exp(accum_out=sums)
           -> scale by 1/sums -> DMA attn rows out.
  tail:  out = concat_h(oT)^T @ w_out + b_out  (K=64 accumulating matmuls).
"""

import sys

if "/opt/trn_rl_repo" not in sys.path:
    sys.path.insert(0, "/opt/trn_rl_repo")

import numpy as np

import concourse.bass as bass
import concourse.bacc as bacc
import concourse.tile as tile
from concourse import mybir
from concourse.bass_utils import run_bass_kernel_spmd

F32 = mybir.dt.float32
F32R = mybir.dt.float32r
U8 = mybir.dt.uint8

N = 2009
D = 512
H = 8
DH = 64
SCALE = DH ** -0.5
R = 510            # query rows per core (4 blocks; last block padded)
P = 128
R0S = [0, 510, 1020, 1530]
AGENT_ROWS = 2000

# pass A query i-tiles (start, nrows)
ITS = [(0, 128), (128, 128), (256, 128), (384, 126)]
# self-score window per tile: (col_start, width); windows always inside [0,510)
WTS = [0, 120, 250, 380]
WS = [144, 144, 144, 130]
# pass A key j-chunks (start, ncols) - psum split in two halves
JA0 = [(0, 512), (512, 512)]          # -> half tile 0  [128,1024]
JA1 = [(1024, 512), (1536, 474)]      # -> half tile 1  [128, 985]
# pass B key j-chunks (start, nrows)
JB = [(128 * m, 128) for m in range(15)] + [(1920, 89)]


def _build_masks(r0: int):
    """Block-diagonal blend masks, all indices local/rotated. float32 {0,1}."""
    mA = np.zeros((4, P, 144), np.uint8)
    mB = np.zeros((4, P, 144), np.uint8)
    for t, (its, nt) in enumerate(ITS):
        wt, w = WTS[t], WS[t]
        q = r0 + its + np.arange(nt)              # global query rows
        kl = wt + np.arange(w)                    # local key cols (<510)
        kg = (r0 + kl) % N                        # global key rows
        qa = np.where(q < AGENT_ROWS, q // 10, -1)
        ka = np.where(kg < AGENT_ROWS, kg // 10, -2)
        mA[t, :nt, :w] = (qa[:, None] == ka[None, :]).astype(np.uint8)
    for c in range(4):
        jcs, njc = JB[c]
        wt, w = WTS[c], WS[c]
        jl = jcs + np.arange(njc)                 # local key rows
        kg = (r0 + jl) % N
        kv = (jl < R) & (kg < AGENT_ROWS)
        ka = np.where(kv, kg // 10, -2)
        ql = wt + np.arange(w)                    # local query cols
        qg = r0 + ql
        qv = (ql < R) & (qg < AGENT_ROWS)
        qa = np.where(qv, qg // 10, -1)
        mB[c, :njc, :w] = (ka[:, None] == qa[None, :]).astype(np.uint8)
    return mA, mB




def build_nc():
    nc = bacc.Bacc("TRN2", target_bir_lowering=False, debug=False)

    xtf = nc.dram_tensor("xtf", [D, N + 1], F32R, kind="ExternalInput")
    wq = nc.dram_tensor("wq", [D, D], F32R, kind="ExternalInput")
    wk = nc.dram_tensor("wk", [D, D], F32R, kind="ExternalInput")
    wv = nc.dram_tensor("wv", [D, D], F32R, kind="ExternalInput")
    wqs = nc.dram_tensor("wqs", [D, D], F32R, kind="ExternalInput")
    wks = nc.dram_tensor("wks", [D, D], F32R, kind="ExternalInput")
    wo = nc.dram_tensor("wo", [D, D], F32R, kind="ExternalInput")
    bo = nc.dram_tensor("bo", [D], F32, kind="ExternalInput")
    one = nc.dram_tensor("one", [P], F32R, kind="ExternalInput")
    mA = nc.dram_tensor("mA", [4, P, 144], U8, kind="ExternalInput")
    mB = nc.dram_tensor("mB", [4, P, 144], U8, kind="ExternalInput")

    attn_p = nc.dram_tensor("attn_p", [H, R, N], F32, kind="ExternalOutput")
    out_p = nc.dram_tensor("out_p", [R, D], F32, kind="ExternalOutput")

    with tile.TileContext(nc) as tc:
        import contextlib

        ctx = contextlib.ExitStack()
        with ctx:
            persist = ctx.enter_context(tc.tile_pool(name="persist", bufs=1))
            wpool = ctx.enter_context(tc.tile_pool(name="wpool", bufs=4))
            smalls = ctx.enter_context(tc.tile_pool(name="smalls", bufs=4))
            drams = ctx.enter_context(tc.tile_pool(name="drams", bufs=2, space="DRAM"))
            big_ps = ctx.enter_context(
                tc.tile_pool(name="big_ps", bufs=1, space="PSUM")
            )
            small_ps = ctx.enter_context(
                tc.tile_pool(name="small_ps", bufs=1, space="PSUM")
            )
            xtf_cm = tc.tile_pool(name="xtf_pool", bufs=1)
            xtf_pool = xtf_cm.__enter__()

            # ---------------- persistent SBUF ----------------
            xtf_sb = [xtf_pool.tile([P, N + 1], F32R, tag=f"xtf{kc}", name=f"xtf{kc}") for kc in range(4)]
            kt_sb = [persist.tile([P, N + 1], F32R, tag=f"kt{p}", name=f"kt{p}") for p in range(4)]
            va_sb = persist.tile([P, 16 * 8 * 65], F32R, tag="va")
            va_v = va_sb.rearrange("p (j h c) -> p j h c", j=16, h=8)
            qt_sb = [persist.tile([P, R], F32R, tag=f"qt{p}", name=f"qt{p}") for p in range(4)]
            qst_sb = [persist.tile([P, R], F32R, tag=f"qst{p}", name=f"qst{p}") for p in range(4)]
            kst_sb = [persist.tile([P, R], F32R, tag=f"kst{p}", name=f"kst{p}") for p in range(4)]
            bo_sb = persist.tile([P, D], F32, tag="bo")
            mA_sb = [persist.tile([P, 144], U8, tag=f"mA{t}", name=f"mAt{t}") for t in range(4)]
            mB_sb = [persist.tile([P, 144], U8, tag=f"mB{t}", name=f"mBt{t}") for t in range(4)]
            ot_sb = [persist.tile([DH, R], F32R, tag=f"ot{h}", name=f"ot{h}") for h in range(H)]
            wo_sb = [persist.tile([DH, D], F32R, tag=f"wo{h}", name=f"wo{h}") for h in range(H)]
            for h in range(H):
                nc.sync.dma_start(out=wo_sb[h][:], in_=wo[h * DH : (h + 1) * DH, :])


            # ---------------- loads ----------------
            for kc in range(4):
                nc.sync.dma_start(
                    out=xtf_sb[kc][:], in_=xtf[kc * P : (kc + 1) * P, :]
                )
            for t in range(4):
                nc.sync.dma_start(out=mA_sb[t][:], in_=mA[t])
                nc.sync.dma_start(out=mB_sb[t][:], in_=mB[t])
            nc.gpsimd.dma_start(
                out=bo_sb[:],
                in_=bass.AP(tensor=bo.ap().tensor, offset=0, ap=[[0, P], [1, D]]),
            )

            # ---------------- projections ----------------
            # kT[pair] [128, N] = wk[:, pair]^T @ xT   (accumulate over 4 kc)
            def proj_dmajor(w_dram, dst_tiles, ncols):
                # dst [128(2 heads), ncols] per pair; rhs = xtf tiles
                nch = [(s, min(512, ncols - s)) for s in range(0, ncols, 512)]
                for pr in range(4):
                    for jcs, njc in nch:
                        ps = small_ps.tile([P, 512], F32, tag="proj", bufs=2)
                        for kc in range(4):
                            wt_t = wpool.tile([P, P], F32R, tag="w")
                            nc.sync.dma_start(
                                out=wt_t[:],
                                in_=w_dram[
                                    kc * P : (kc + 1) * P, pr * P : (pr + 1) * P
                                ],
                            )
                            nc.tensor.matmul(
                                out=ps[:, :njc],
                                lhsT=(wt_t[:]),
                                rhs=(xtf_sb[kc][:, jcs : jcs + njc]),
                                start=(kc == 0),
                                stop=(kc == 3),
                            )
                        nc.vector.tensor_copy(
                            out=dst_tiles[pr][:, jcs : jcs + njc], in_=ps[:, :njc]
                        )

            proj_dmajor(wk, kt_sb, N + 1)
            proj_dmajor(wq, qt_sb, R)
            proj_dmajor(wqs, qst_sb, R)
            proj_dmajor(wks, kst_sb, R)

            # v (j-major, ones-augmented): va[j, h, 0:64]=v, va[j, h, 64]=1
            for jc in range(16):
                jcs, njc = JB[jc]
                ps = small_ps.tile([P, 512], F32, tag="proj", bufs=2)
                for kc in range(4):
                    wt_t = wpool.tile([P, D], F32R, tag="wv")
                    nc.sync.dma_start(out=wt_t[:], in_=wv[kc * P : (kc + 1) * P, :])
                    nc.tensor.matmul(
                        out=ps[:njc, :],
                        lhsT=(xtf_sb[kc][:, jcs : jcs + njc]),
                        rhs=(wt_t[:]),
                        start=(kc == 0),
                        stop=(kc == 3),
                    )
                psv = ps.rearrange("p (h c) -> p h c", h=8)
                nc.vector.tensor_copy(
                    out=va_v[:njc, jc, :, 0:64], in_=psv[:njc, :, :]
                )

            xtf_cm.__exit__(None, None, None)
            apool = ctx.enter_context(tc.tile_pool(name="apool", bufs=2))
            etpool = ctx.enter_context(tc.tile_pool(name="etpool", bufs=2))

            ones_st = smalls.tile([P, P], F32R, tag="ones", bufs=1)
            nc.gpsimd.dma_start(
                out=ones_st[:],
                in_=bass.AP(
                    tensor=one.ap().tensor, offset=0, ap=[[0, P], [1, P]]
                ),
            )
            nc.vector.tensor_copy(
                out=bass.AP(
                    tensor=va_sb.tensor,
                    offset=va_sb.offset + 64,
                    ap=[[16 * 8 * 65, P], [65, 16 * 8], [1, 1]],
                ),
                in_=ones_st[:, 0 : 16 * 8].rearrange("p (a b) -> p a b", b=1),
            )

            # ---------------- per-head passes ----------------
            for h in range(H):
                pr, po = h // 2, (h % 2) * DH

                # ---- pass B: [key, query] layout; accumulate outT ----
                otp = small_ps.tile([DH + 1, R], F32, tag="outT", bufs=1)
                for jc in range(16):
                    jcs, njc = JB[jc]
                    dtp = small_ps.tile([P, R], F32, tag="dotsT", bufs=2)
                    nc.tensor.matmul(
                        out=dtp[:njc, :],
                        lhsT=(kt_sb[pr][po : po + DH, jcs : jcs + njc]),
                        rhs=(qt_sb[pr][po : po + DH, :]),
                        start=True,
                        stop=True,
                    )
                    if jc < 4:
                        wt, w = WTS[jc], WS[jc]
                        njs = min(njc, R - jcs)
                        sfp = small_ps.tile([P, 144], F32, tag="self", bufs=1)
                        nc.tensor.matmul(
                            out=sfp[:njs, :w],
                            lhsT=(kst_sb[pr][po : po + DH, jcs : jcs + njs]),
                            rhs=(qst_sb[pr][po : po + DH, wt : wt + w]),
                            start=True,
                            stop=True,
                        )
                        nc.vector.copy_predicated(
                            out=dtp[:njs, wt : wt + w],
                            mask=mB_sb[jc][:njs, :w],
                            data=sfp[:njs, :w],
                        )
                    et = etpool.tile([P, R], F32R, tag="et")
                    nc.scalar.activation(
                        out=et[:njc, :],
                        in_=dtp[:njc, :],
                        func=mybir.ActivationFunctionType.Exp,
                        scale=SCALE,
                    )
                    nc.tensor.matmul(
                        out=otp[:, :],
                        lhsT=(va_v[:njc, jc, h, :]),
                        rhs=(et[:njc, :]),
                        start=(jc == 0),
                        stop=(jc == 15),
                    )

                # denominators -> reciprocal -> broadcast to 64 partitions
                rs = smalls.tile([1, R], F32, tag="rs", bufs=2)
                nc.vector.reciprocal(out=rs[:], in_=otp[DH : DH + 1, :])
                rsd = drams.tile([1, R], F32, tag="rsd", bufs=2)
                nc.sync.dma_start(out=rsd[:], in_=rs[:])
                rb = smalls.tile([DH, R], F32, tag="recb", bufs=2)
                nc.gpsimd.dma_start(
                    out=rb[:],
                    in_=bass.AP(
                        tensor=rsd.tensor,
                        offset=rsd.offset,
                        ap=[[0, DH]] + [list(p) for p in rsd.ap[1:]],
                    ),
                )
                nc.vector.tensor_mul(
                    out=ot_sb[h][:], in0=otp[0:DH, :], in1=rb[:]
                )

                # ---- pass A: [query, key] layout; write attn rows ----
                for t, (its, nt) in enumerate(ITS):
                    at = apool.tile([P, 2010], F32, tag="attn")
                    nc.vector.memset(at[:nt, 2009:2010], 1.0)
                    acc = smalls.tile([P, 2], F32, tag="acc")
                    for half, chunks in enumerate((JA0, JA1)):
                        hs = chunks[0][0]
                        # exclude the padded key column 2009 from exp/accum
                        hw_ = min(sum(c[1] for c in chunks), N - hs)
                        dp = big_ps.tile([P, 1024], F32, tag="dots", bufs=1)
                        for jcs, njc in chunks:
                            nc.tensor.matmul(
                                out=dp[:nt, jcs - hs : jcs - hs + njc],
                                lhsT=(qt_sb[pr][po : po + DH, its : its + nt]),
                                rhs=(kt_sb[pr][po : po + DH, jcs : jcs + njc]),
                                start=True,
                                stop=True,
                            )
                        if half == 0:
                            wt, w = WTS[t], WS[t]
                            sfp = small_ps.tile([P, 144], F32, tag="self", bufs=1)
                            nc.tensor.matmul(
                                out=sfp[:nt, :w],
                                lhsT=(qst_sb[pr][po : po + DH, its : its + nt]),
                                rhs=(kst_sb[pr][po : po + DH, wt : wt + w]),
                                start=True,
                                stop=True,
                            )
                            nc.vector.copy_predicated(
                                out=dp[:nt, wt : wt + w],
                                mask=mA_sb[t][:nt, :w],
                                data=sfp[:nt, :w],
                            )
                        nc.scalar.activation(
                            out=at[:nt, hs : hs + hw_],
                            in_=dp[:nt, :hw_],
                            func=mybir.ActivationFunctionType.Exp,
                            scale=SCALE,
                            accum_out=acc[:nt, half : half + 1],
                        )
                    ssum = smalls.tile([P, 1], F32, tag="ssum")
                    nc.vector.tensor_add(
                        out=ssum[:nt, :], in0=acc[:nt, 0:1], in1=acc[:nt, 1:2]
                    )
                    nc.vector.reciprocal(out=ssum[:nt, :], in_=ssum[:nt, :])
                    nc.vector.tensor_scalar_mul(
                        out=at[:nt, :], in0=at[:nt, :], scalar1=ssum[:nt, :]
                    )
                    nc.sync.dma_start(
                        out=attn_p[h, its : its + nt, :], in_=at[:nt, 0:2009]
                    )

            # ---------------- output projection ----------------
            for t, (its, nt) in enumerate(ITS):
                ops = small_ps.tile([P, 512], F32, tag="dotsT", bufs=2, name=f"ops{t}")
                for h in range(H):
                    nc.tensor.matmul(
                        out=ops[:nt, :],
                        lhsT=(ot_sb[h][:, its : its + nt]),
                        rhs=(wo_sb[h][:]),
                        start=(h == 0),
                        stop=(h == 7),
                    )
                osb = smalls.tile([P, D], F32, tag="osb", bufs=2, name=f"osb{t}")
                nc.vector.tensor_add(
                    out=osb[:nt, :], in0=ops[:nt, :], in1=bo_sb[:nt, :]
                )
                nc.sync.dma_start(out=out_p[its : its + nt, :], in_=osb[:nt, :])

    nc.compile()
    return nc


_NC = None


def _get_nc():
    global _NC
    if _NC is None:
        _NC = build_nc()
    return _NC


def make_in_maps(x, w_qkv, w_qk_self, w_out, b_out):
    x = np.asarray(x, np.float32)
    w_qkv = np.asarray(w_qkv, np.float32)
    w_qk_self = np.asarray(w_qk_self, np.float32)
    w_out = np.ascontiguousarray(np.asarray(w_out, np.float32))
    b_out = np.ascontiguousarray(np.asarray(b_out, np.float32))
    wq = np.ascontiguousarray(w_qkv[:, 0:512])
    wk = np.ascontiguousarray(w_qkv[:, 512:1024])
    wv = np.ascontiguousarray(w_qkv[:, 1024:1536])
    wqs = np.ascontiguousarray(w_qk_self[:, 0:512])
    wks = np.ascontiguousarray(w_qk_self[:, 512:1024])
    in_maps = []
    for c in range(8):
        b, blk = c // 4, c % 4
        r0 = R0S[blk]
        xrot = np.roll(x[b], -r0, axis=0)
        xtf = np.zeros((D, N + 1), np.float32)
        xtf[:, :N] = xrot.T
        mAv, mBv = _build_masks(r0)
        in_maps.append(
            dict(
                xtf=xtf, wq=wq, wk=wk, wv=wv, wqs=wqs, wks=wks,
                wo=w_out, bo=b_out, mA=mAv, mB=mBv, one=np.ones(128, np.float32),
            )
        )
    return in_maps


def unshard(results):
    out = np.zeros((2, N, D), np.float32)
    attn = np.zeros((2, H, N, N), np.float32)
    for c in range(8):
        b, blk = c // 4, c % 4
        r0 = R0S[blk]
        nv = min(R, N - r0)
        ap = results[c]["attn_p"]
        attn[b, :, r0 : r0 + nv, :] = np.roll(ap[:, :nv, :], r0, axis=-1)
        out[b, r0 : r0 + nv, :] = results[c]["out_p"][:nv]
    return out, attn


def kernel(x, w_qkv, w_qk_self, w_out, b_out):
    nc = _get_nc()
    in_maps = make_in_maps(x, w_qkv, w_qk_self, w_out, b_out)
    res = run_bass_kernel_spmd(nc, in_maps, core_ids=list(range(8)))
    return unshard(res.results)


# revision 19
# speedup vs baseline: 3.7573x; 3.7573x over previous
"""AgentAwareAttention Trainium2 kernel (8 NeuronCores, SPMD).

Sharding: core c -> batch b=c//4, query-row block r0 = 510*(c%4).
Keys are ROTATED per core by r0 so every SBUF offset is core-independent
(pure SPMD).  Host unshard = np.roll (inverse rotation) + concat.

Per core (all 8 heads, query rows [r0, r0+510), keys all 2009 rotated):
  phase 0: load xT (pre-transposed on host), project kT/ksT/qT/qsT (d-major)
           and v (j-major, ones-augmented 65th column for row sums).
  per head:
    pass B ([key, query] layout): dotsT = k^T q tiles -> blend block-diag
           self scores (copy_predicated w/ host mask) -> exp -> accumulate
           outT[65,510] = v_aug^T @ E^T  (row 64 = softmax denominators)
           -> outT scaled by 1/sums (broadcast DMA) -> SBUF per-head oT.
  pass A ([query, key] layout): dots tiles -> blend -> exp(accum_out=sums)
           -> scale by 1/sums -> DMA attn rows out.
  tail:  out = concat_h(oT)^T @ w_out + b_out  (K=64 accumulating matmuls).
"""

import sys

if "/opt/trn_rl_repo" not in sys.path:
    sys.path.insert(0, "/opt/trn_rl_repo")

import numpy as np

import concourse.bass as bass
import concourse.bacc as bacc
import concourse.tile as tile
from concourse import mybir
from concourse.bass_utils import run_bass_kernel_spmd

F32 = mybir.dt.float32
F32R = mybir.dt.float32r
U8 = mybir.dt.uint8

N = 2009
D = 512
H = 8
DH = 64
SCALE = DH ** -0.5
R = 510            # query rows per core (4 blocks; last block padded)
P = 128
R0S = [0, 510, 1020, 1530]
AGENT_ROWS = 2000

# pass A query i-tiles (start, nrows)
ITS = [(0, 128), (128, 128), (256, 128), (384, 126)]
# self-score window per tile: (col_start, width); windows always inside [0,510)
WTS = [0, 120, 250, 380]
WS = [144, 144, 144, 130]
# pass A key j-chunks (start, ncols) - psum split in two halves
JA0 = [(0, 512), (512, 512)]          # -> half tile 0  [128,1024]
JA1 = [(1024, 512), (1536, 474)]      # -> half tile 1  [128, 985]
# pass B key j-chunks (start, nrows)
JB = [(128 * m, 128) for m in range(15)] + [(1920, 89)]


def _build_masks(r0: int):
    """Block-diagonal blend masks, all indices local/rotated. float32 {0,1}."""
    mA = np.zeros((4, P, 144), np.uint8)
    mB = np.zeros((4, P, 144), np.uint8)
    for t, (its, nt) in enumerate(ITS):
        wt, w = WTS[t], WS[t]
        q = r0 + its + np.arange(nt)              # global query rows
        kl = wt + np.arange(w)                    # local key cols (<510)
        kg = (r0 + kl) % N                        # global key rows
        qa = np.where(q < AGENT_ROWS, q // 10, -1)
        ka = np.where(kg < AGENT_ROWS, kg // 10, -2)
        mA[t, :nt, :w] = (qa[:, None] == ka[None, :]).astype(np.uint8)
    for c in range(4):
        jcs, njc = JB[c]
        wt, w = WTS[c], WS[c]
        jl = jcs + np.arange(njc)                 # local key rows
        kg = (r0 + jl) % N
        kv = (jl < R) & (kg < AGENT_ROWS)
        ka = np.where(kv, kg // 10, -2)
        ql = wt + np.arange(w)                    # local query cols
        qg = r0 + ql
        qv = (ql < R) & (qg < AGENT_ROWS)
        qa = np.where(qv, qg // 10, -1)
        mB[c, :njc, :w] = (ka[:, None] == qa[None, :]).astype(np.uint8)
    return mA, mB




def build_nc(perf_probe=False):
    nc = bacc.Bacc("TRN2", target_bir_lowering=False, debug=False)

    xtf = nc.dram_tensor("xtf", [D, N + 1], F32R, kind="ExternalInput")
    wq = nc.dram_tensor("wq", [D, D], F32R, kind="ExternalInput")
    wk = nc.dram_tensor("wk", [D, D], F32R, kind="ExternalInput")
    wv = nc.dram_tensor("wv", [D, D], F32R, kind="ExternalInput")
    wqs = nc.dram_tensor("wqs", [D, D], F32R, kind="ExternalInput")
    wks = nc.dram_tensor("wks", [D, D], F32R, kind="ExternalInput")
    wo = nc.dram_tensor("wo", [D, D], F32R, kind="ExternalInput")
    bo = nc.dram_tensor("bo", [D], F32, kind="ExternalInput")
    one = nc.dram_tensor("one", [P], F32R, kind="ExternalInput")
    mA = nc.dram_tensor("mA", [4, P, 144], U8, kind="ExternalInput")
    mB = nc.dram_tensor("mB", [4, P, 144], U8, kind="ExternalInput")

    attn_kind = "Internal" if perf_probe else "ExternalOutput"
    attn_p = nc.dram_tensor("attn_p", [H, R, N], F32, kind=attn_kind)
    out_p = nc.dram_tensor("out_p", [R, D], F32, kind="ExternalOutput")

    with tile.TileContext(nc) as tc:
        import contextlib

        ctx = contextlib.ExitStack()
        with ctx:
            persist = ctx.enter_context(tc.tile_pool(name="persist", bufs=1))
            wpool = ctx.enter_context(tc.tile_pool(name="wpool", bufs=4))
            smalls = ctx.enter_context(tc.tile_pool(name="smalls", bufs=4))
            drams = ctx.enter_context(tc.tile_pool(name="drams", bufs=2, space="DRAM"))
            big_ps = ctx.enter_context(
                tc.tile_pool(name="big_ps", bufs=1, space="PSUM")
            )
            small_ps = ctx.enter_context(
                tc.tile_pool(name="small_ps", bufs=1, space="PSUM")
            )
            xtf_cm = tc.tile_pool(name="xtf_pool", bufs=1)
            xtf_pool = xtf_cm.__enter__()

            # ---------------- persistent SBUF ----------------
            xtf_sb = [xtf_pool.tile([P, N + 1], F32R, tag=f"xtf{kc}", name=f"xtf{kc}") for kc in range(4)]
            kt_sb = [persist.tile([P, N + 1], F32R, tag=f"kt{p}", name=f"kt{p}") for p in range(4)]
            va_sb = persist.tile([P, 16 * 8 * 65], F32R, tag="va")
            va_v = va_sb.rearrange("p (j h c) -> p j h c", j=16, h=8)
            qt_sb = [persist.tile([P, R], F32R, tag=f"qt{p}", name=f"qt{p}") for p in range(4)]
            qst_sb = [persist.tile([P, R], F32R, tag=f"qst{p}", name=f"qst{p}") for p in range(4)]
            kst_sb = [persist.tile([P, R], F32R, tag=f"kst{p}", name=f"kst{p}") for p in range(4)]
            bo_sb = persist.tile([P, D], F32, tag="bo")
            mA_sb = [persist.tile([P, 144], U8, tag=f"mA{t}", name=f"mAt{t}") for t in range(4)]
            mB_sb = [persist.tile([P, 144], U8, tag=f"mB{t}", name=f"mBt{t}") for t in range(4)]
            ot_sb = [persist.tile([DH, R], F32R, tag=f"ot{h}", name=f"ot{h}") for h in range(H)]
            wo_sb = [persist.tile([DH, D], F32R, tag=f"wo{h}", name=f"wo{h}") for h in range(H)]
            for h in range(H):
                nc.sync.dma_start(out=wo_sb[h][:], in_=wo[h * DH : (h + 1) * DH, :])


            # ---------------- loads ----------------
            for kc in range(4):
                nc.sync.dma_start(
                    out=xtf_sb[kc][:], in_=xtf[kc * P : (kc + 1) * P, :]
                )
            for t in range(4):
                nc.sync.dma_start(out=mA_sb[t][:], in_=mA[t])
                nc.sync.dma_start(out=mB_sb[t][:], in_=mB[t])
            nc.gpsimd.dma_start(
                out=bo_sb[:],
                in_=bass.AP(tensor=bo.ap().tensor, offset=0, ap=[[0, P], [1, D]]),
            )

            # ---------------- projections ----------------
            # kT[pair] [128, N] = wk[:, pair]^T @ xT   (accumulate over 4 kc)
            def proj_dmajor(w_dram, dst_tiles, ncols):
                # dst [128(2 heads), ncols] per pair; rhs = xtf tiles
                nch = [(s, min(512, ncols - s)) for s in range(0, ncols, 512)]
                for pr in range(4):
                    for jcs, njc in nch:
                        ps = small_ps.tile([P, 512], F32, tag="proj", bufs=2)
                        for kc in range(4):
                            wt_t = wpool.tile([P, P], F32R, tag="w")
                            nc.sync.dma_start(
                                out=wt_t[:],
                                in_=w_dram[
                                    kc * P : (kc + 1) * P, pr * P : (pr + 1) * P
                                ],
                            )
                            nc.tensor.matmul(
                                out=ps[:, :njc],
                                lhsT=(wt_t[:]),
                                rhs=(xtf_sb[kc][:, jcs : jcs + njc]),
                                start=(kc == 0),
                                stop=(kc == 3),
                            )
                        nc.vector.tensor_copy(
                            out=dst_tiles[pr][:, jcs : jcs + njc], in_=ps[:, :njc]
                        )

            proj_dmajor(wk, kt_sb, N + 1)
            proj_dmajor(wq, qt_sb, R)
            proj_dmajor(wqs, qst_sb, R)
            proj_dmajor(wks, kst_sb, R)

            # v (j-major, ones-augmented): va[j, h, 0:64]=v, va[j, h, 64]=1
            for jc in range(16):
                jcs, njc = JB[jc]
                ps = small_ps.tile([P, 512], F32, tag="proj", bufs=2)
                for kc in range(4):
                    wt_t = wpool.tile([P, D], F32R, tag="wv")
                    nc.sync.dma_start(out=wt_t[:], in_=wv[kc * P : (kc + 1) * P, :])
                    nc.tensor.matmul(
                        out=ps[:njc, :],
                        lhsT=(xtf_sb[kc][:, jcs : jcs + njc]),
                        rhs=(wt_t[:]),
                        start=(kc == 0),
                        stop=(kc == 3),
                    )
                psv = ps.rearrange("p (h c) -> p h c", h=8)
                nc.vector.tensor_copy(
                    out=va_v[:njc, jc, :, 0:64], in_=psv[:njc, :, :]
                )

            xtf_cm.__exit__(None, None, None)
            apool = ctx.enter_context(tc.tile_pool(name="apool", bufs=2))
            etpool = ctx.enter_context(tc.tile_pool(name="etpool", bufs=2))

            ones_st = smalls.tile([P, P], F32R, tag="ones", bufs=1)
            nc.gpsimd.dma_start(
                out=ones_st[:],
                in_=bass.AP(
                    tensor=one.ap().tensor, offset=0, ap=[[0, P], [1, P]]
                ),
            )
            nc.vector.tensor_copy(
                out=bass.AP(
                    tensor=va_sb.tensor,
                    offset=va_sb.offset + 64,
                    ap=[[16 * 8 * 65, P], [65, 16 * 8], [1, 1]],
                ),
                in_=ones_st[:, 0 : 16 * 8].rearrange("p (a b) -> p a b", b=1),
            )

            # ---------------- per-head passes ----------------
            for h in range(H):
                pr, po = h // 2, (h % 2) * DH

                # ---- pass B: [key, query] layout; accumulate outT ----
                otp = small_ps.tile([DH + 1, R], F32, tag="outT", bufs=1)
                for jc in range(16):
                    jcs, njc = JB[jc]
                    dtp = small_ps.tile([P, R], F32, tag="dotsT", bufs=2)
                    nc.tensor.matmul(
                        out=dtp[:njc, :],
                        lhsT=(kt_sb[pr][po : po + DH, jcs : jcs + njc]),
                        rhs=(qt_sb[pr][po : po + DH, :]),
                        start=True,
                        stop=True,
                    )
                    if jc < 4:
                        wt, w = WTS[jc], WS[jc]
                        njs = min(njc, R - jcs)
                        sfp = small_ps.tile([P, 144], F32, tag="self", bufs=1)
                        nc.tensor.matmul(
                            out=sfp[:njs, :w],
                            lhsT=(kst_sb[pr][po : po + DH, jcs : jcs + njs]),
                            rhs=(qst_sb[pr][po : po + DH, wt : wt + w]),
                            start=True,
                            stop=True,
                        )
                        nc.vector.copy_predicated(
                            out=dtp[:njs, wt : wt + w],
                            mask=mB_sb[jc][:njs, :w],
                            data=sfp[:njs, :w],
                        )
                    et = etpool.tile([P, R], F32R, tag="et")
                    nc.scalar.activation(
                        out=et[:njc, :],
                        in_=dtp[:njc, :],
                        func=mybir.ActivationFunctionType.Exp,
                        scale=SCALE,
                    )
                    nc.tensor.matmul(
                        out=otp[:, :],
                        lhsT=(va_v[:njc, jc, h, :]),
                        rhs=(et[:njc, :]),
                        start=(jc == 0),
                        stop=(jc == 15),
                    )

                # denominators -> reciprocal -> broadcast to 64 partitions
                rs = smalls.tile([1, R], F32, tag="rs", bufs=2)
                nc.vector.reciprocal(out=rs[:], in_=otp[DH : DH + 1, :])
                rsd = drams.tile([1, R], F32, tag="rsd", bufs=2)
                nc.sync.dma_start(out=rsd[:], in_=rs[:])
                rb = smalls.tile([DH, R], F32, tag="recb", bufs=2)
                nc.gpsimd.dma_start(
                    out=rb[:],
                    in_=bass.AP(
                        tensor=rsd.tensor,
                        offset=rsd.offset,
                        ap=[[0, DH]] + [list(p) for p in rsd.ap[1:]],
                    ),
                )
                nc.vector.tensor_mul(
                    out=ot_sb[h][:], in0=otp[0:DH, :], in1=rb[:]
                )

                # ---- pass A: [query, key] layout; write attn rows ----
                for t, (its, nt) in enumerate(ITS):
                    at = apool.tile([P, 2010], F32, tag="attn")
                    nc.vector.memset(at[:nt, 2009:2010], 1.0)
                    acc = smalls.tile([P, 2], F32, tag="acc")
                    for half, chunks in enumerate((JA0, JA1)):
                        hs = chunks[0][0]
                        # exclude the padded key column 2009 from exp/accum
                        hw_ = min(sum(c[1] for c in chunks), N - hs)
                        dp = big_ps.tile([P, 1024], F32, tag="dots", bufs=1)
                        for jcs, njc in chunks:
                            nc.tensor.matmul(
                                out=dp[:nt, jcs - hs : jcs - hs + njc],
                                lhsT=(qt_sb[pr][po : po + DH, its : its + nt]),
                                rhs=(kt_sb[pr][po : po + DH, jcs : jcs + njc]),
                                start=True,
                                stop=True,
                            )
                        if half == 0:
                            wt, w = WTS[t], WS[t]
                            sfp = small_ps.tile([P, 144], F32, tag="self", bufs=1)
                            nc.tensor.matmul(
                                out=sfp[:nt, :w],
                                lhsT=(qst_sb[pr][po : po + DH, its : its + nt]),
                                rhs=(kst_sb[pr][po : po + DH, wt : wt + w]),
                                start=True,
                                stop=True,
                            )
                            nc.vector.copy_predicated(
                                out=dp[:nt, wt : wt + w],
                                mask=mA_sb[t][:nt, :w],
                                data=sfp[:nt, :w],
                            )
                        nc.scalar.activation(
                            out=at[:nt, hs : hs + hw_],
                            in_=dp[:nt, :hw_],
                            func=mybir.ActivationFunctionType.Exp,
                            scale=SCALE,
                            accum_out=acc[:nt, half : half + 1],
                        )
                    ssum = smalls.tile([P, 1], F32, tag="ssum")
                    nc.vector.tensor_add(
                        out=ssum[:nt, :], in0=acc[:nt, 0:1], in1=acc[:nt, 1:2]
                    )
                    nc.vector.reciprocal(out=ssum[:nt, :], in_=ssum[:nt, :])
                    nc.vector.tensor_scalar_mul(
                        out=at[:nt, :], in0=at[:nt, :], scalar1=ssum[:nt, :]
                    )
                    nc.sync.dma_start(
                        out=attn_p[h, its : its + nt, :], in_=at[:nt, 0:2009]
                    )

            # ---------------- output projection ----------------
            for t, (its, nt) in enumerate(ITS):
                ops = small_ps.tile([P, 512], F32, tag="dotsT", bufs=2, name=f"ops{t}")
                for h in range(H):
                    nc.tensor.matmul(
                        out=ops[:nt, :],
                        lhsT=(ot_sb[h][:, its : its + nt]),
                        rhs=(wo_sb[h][:]),
                        start=(h == 0),
                        stop=(h == 7),
                    )
                osb = smalls.tile([P, D], F32, tag="osb", bufs=2, name=f"osb{t}")
                nc.vector.tensor_add(
                    out=osb[:nt, :], in0=ops[:nt, :], in1=bo_sb[:nt, :]
                )
                nc.sync.dma_start(out=out_p[its : its + nt, :], in_=osb[:nt, :])

    nc.compile()
    return nc


_NC = None


def _get_nc():
    global _NC
    if _NC is None:
        _NC = build_nc()
    return _NC


def make_in_maps(x, w_qkv, w_qk_self, w_out, b_out):
    x = np.asarray(x, np.float32)
    w_qkv = np.asarray(w_qkv, np.float32)
    w_qk_self = np.asarray(w_qk_self, np.float32)
    w_out = np.ascontiguousarray(np.asarray(w_out, np.float32))
    b_out = np.ascontiguousarray(np.asarray(b_out, np.float32))
    wq = np.ascontiguousarray(w_qkv[:, 0:512])
    wk = np.ascontiguousarray(w_qkv[:, 512:1024])
    wv = np.ascontiguousarray(w_qkv[:, 1024:1536])
    wqs = np.ascontiguousarray(w_qk_self[:, 0:512])
    wks = np.ascontiguousarray(w_qk_self[:, 512:1024])
    in_maps = []
    for c in range(8):
        b, blk = c // 4, c % 4
        r0 = R0S[blk]
        xrot = np.roll(x[b], -r0, axis=0)
        xtf = np.zeros((D, N + 1), np.float32)
        xtf[:, :N] = xrot.T
        mAv, mBv = _build_masks(r0)
        in_maps.append(
            dict(
                xtf=xtf, wq=wq, wk=wk, wv=wv, wqs=wqs, wks=wks,
                wo=w_out, bo=b_out, mA=mAv, mB=mBv, one=np.ones(128, np.float32),
            )
        )
    return in_maps


def unshard(results):
    out = np.zeros((2, N, D), np.float32)
    attn = np.zeros((2, H, N, N), np.float32)
    for c in range(8):
        b, blk = c // 4, c % 4
        r0 = R0S[blk]
        nv = min(R, N - r0)
        ap = results[c]["attn_p"]
        attn[b, :, r0 : r0 + nv, :] = np.roll(ap[:, :nv, :], r0, axis=-1)
        out[b, r0 : r0 + nv, :] = results[c]["out_p"][:nv]
    return out, attn


def kernel(x, w_qkv, w_qk_self, w_out, b_out):
    nc = _get_nc()
    in_maps = make_in_maps(x, w_qkv, w_qk_self, w_out, b_out)
    res = run_bass_kernel_spmd(nc, in_maps, core_ids=list(range(8)))
    return unshard(res.results)


# revision 37
# speedup vs baseline: 88.3921x; 23.5251x over previous
"""AgentAwareAttention Trainium2 kernel (8 NeuronCores, SPMD).

Sharding: core c -> batch b=c//4, query-row block r0 = 510*(c%4).
Keys are ROTATED per core by r0 so every SBUF offset is core-independent
(pure SPMD).  Host unshard = np.roll (inverse rotation) + concat.

Per core (all 8 heads, query rows [r0, r0+510), keys all 2009 rotated):
  phase 0: load xT (pre-transposed on host), project kT/ksT/qT/qsT (d-major)
           and v (j-major, ones-augmented 65th column for row sums).
  per head:
    pass B ([key, query] layout): dotsT = k^T q tiles -> blend block-diag
           self scores (copy_predicated w/ host mask) -> exp -> accumulate
           outT[65,510] = v_aug^T @ E^T  (row 64 = softmax denominators)
           -> outT scaled by 1/sums (broadcast DMA) -> SBUF per-head oT.
  pass A ([query, key] layout): dots tiles -> blend -> exp(accum_out=sums)
           -> scale by 1/sums -> DMA attn rows out.
  tail:  out = concat_h(oT)^T @ w_out + b_out  (K=64 accumulating matmuls).
"""

import sys

if "/opt/trn_rl_repo" not in sys.path:
    sys.path.insert(0, "/opt/trn_rl_repo")

import numpy as np

import concourse.bass as bass
import concourse.bacc as bacc
import concourse.tile as tile
from concourse import mybir
from concourse.bass_utils import run_bass_kernel_spmd

F32 = mybir.dt.float32
F32R = mybir.dt.float32r
U8 = mybir.dt.uint8

N = 2009
D = 512
H = 8
DH = 64
SCALE = DH ** -0.5
R = 510            # query rows per core (4 blocks; last block padded)
P = 128
R0S = [0, 510, 1020, 1530]
AGENT_ROWS = 2000

# pass A query i-tiles (start, nrows)
ITS = [(0, 128), (128, 128), (256, 128), (384, 126)]
# self-score window per tile: (col_start, width); windows always inside [0,510)
WTS = [0, 120, 250, 380]
WS = [144, 144, 144, 130]
# pass A key j-chunks (start, ncols) - psum split in two halves
JA0 = [(0, 512), (512, 512)]          # -> half tile 0  [128,1024]
JA1 = [(1024, 512), (1536, 474)]      # -> half tile 1  [128, 985]
# pass B key j-chunks (start, nrows)
JB = [(128 * m, 128) for m in range(15)] + [(1920, 89)]


def _build_masks(r0: int):
    """Block-diagonal blend masks, all indices local/rotated. float32 {0,1}."""
    mA = np.zeros((4, P, 144), np.uint8)
    mB = np.zeros((4, P, 144), np.uint8)
    for t, (its, nt) in enumerate(ITS):
        wt, w = WTS[t], WS[t]
        q = r0 + its + np.arange(nt)              # global query rows
        kl = wt + np.arange(w)                    # local key cols (<510)
        kg = (r0 + kl) % N                        # global key rows
        qa = np.where(q < AGENT_ROWS, q // 10, -1)
        ka = np.where(kg < AGENT_ROWS, kg // 10, -2)
        mA[t, :nt, :w] = (qa[:, None] == ka[None, :]).astype(np.uint8)
    for c in range(4):
        jcs, njc = JB[c]
        wt, w = WTS[c], WS[c]
        jl = jcs + np.arange(njc)                 # local key rows
        kg = (r0 + jl) % N
        kv = (jl < R) & (kg < AGENT_ROWS)
        ka = np.where(kv, kg // 10, -2)
        ql = wt + np.arange(w)                    # local query cols
        qg = r0 + ql
        qv = (ql < R) & (qg < AGENT_ROWS)
        qa = np.where(qv, qg // 10, -1)
        mB[c, :njc, :w] = (ka[:, None] == qa[None, :]).astype(np.uint8)
    return mA, mB




def build_nc(perf_probe=False):
    nc = bacc.Bacc("TRN2", target_bir_lowering=False, debug=False)

    xtf = nc.dram_tensor("xtf", [D, N + 1], F32R, kind="ExternalInput")
    wq = nc.dram_tensor("wq", [D, D], F32R, kind="ExternalInput")
    wk = nc.dram_tensor("wk", [D, D], F32R, kind="ExternalInput")
    wv = nc.dram_tensor("wv", [D, D], F32R, kind="ExternalInput")
    wqs = nc.dram_tensor("wqs", [D, D], F32R, kind="ExternalInput")
    wks = nc.dram_tensor("wks", [D, D], F32R, kind="ExternalInput")
    wo = nc.dram_tensor("wo", [D, D], F32R, kind="ExternalInput")
    bo = nc.dram_tensor("bo", [D], F32, kind="ExternalInput")
    one = nc.dram_tensor("one", [P], F32R, kind="ExternalInput")
    mA = nc.dram_tensor("mA", [4, P, 144], U8, kind="ExternalInput")
    mB = nc.dram_tensor("mB", [4, P, 144], U8, kind="ExternalInput")

    attn_kind = "Internal" if perf_probe else "ExternalOutput"
    attn_p = nc.dram_tensor("attn_p", [H, R, N], F32, kind=attn_kind)
    out_p = nc.dram_tensor("out_p", [R, D], F32, kind="ExternalOutput")

    with tile.TileContext(nc) as tc:
        import contextlib

        ctx = contextlib.ExitStack()
        with ctx:
            persist = ctx.enter_context(tc.tile_pool(name="persist", bufs=1))
            wpool = ctx.enter_context(tc.tile_pool(name="wpool", bufs=4))
            smalls = ctx.enter_context(tc.tile_pool(name="smalls", bufs=4))
            drams = ctx.enter_context(tc.tile_pool(name="drams", bufs=2, space="DRAM"))
            proj_cm = tc.tile_pool(name="proj_ps", bufs=3, space="PSUM")
            proj_ps = proj_cm.__enter__()

            # ---------------- persistent SBUF ----------------
            xtf_sb = [persist.tile([P, N + 1], F32R, tag=f"xtf{kc}", name=f"xtf{kc}") for kc in range(4)]
            kt_sb = [persist.tile([P, N + 1], F32R, tag=f"kt{p}", name=f"kt{p}") for p in range(4)]
            va_sb = persist.tile([P, 16 * 8 * 65], F32R, tag="va")
            va_v = va_sb.rearrange("p (j h c) -> p j h c", j=16, h=8)
            qt_sb = [persist.tile([P, 512], F32R, tag=f"qt{p}", name=f"qt{p}") for p in range(4)]
            qst_sb = [persist.tile([P, 512], F32R, tag=f"qst{p}", name=f"qst{p}") for p in range(4)]
            kst_sb = [persist.tile([P, 512], F32R, tag=f"kst{p}", name=f"kst{p}") for p in range(4)]
            bo_sb = persist.tile([P, D], F32, tag="bo")
            mA_sb = [persist.tile([P, 144], U8, tag=f"mA{t}", name=f"mAt{t}") for t in range(4)]
            mB_sb = [persist.tile([P, 144], U8, tag=f"mB{t}", name=f"mBt{t}") for t in range(4)]
            ot_sb = [persist.tile([DH, R], F32R, tag=f"ot{h}", name=f"ot{h}") for h in range(H)]


            # ---------------- loads ----------------
            xtf_dma_engines = [nc.sync, nc.scalar, nc.gpsimd, nc.gpsimd]
            for kc in range(4):
                xtf_dma_engines[kc].dma_start(
                    out=xtf_sb[kc][:], in_=xtf[kc * P : (kc + 1) * P, :]
                )
            for t in range(4):
                nc.sync.dma_start(out=mA_sb[t][:], in_=mA[t])
                nc.sync.dma_start(out=mB_sb[t][:], in_=mB[t])
            nc.gpsimd.dma_start(
                out=bo_sb[:],
                in_=bass.AP(tensor=bo.ap().tensor, offset=0, ap=[[0, P], [1, D]]),
            )

            # ---------------- projections ----------------
            # kT[pair] [128, N] = wk[:, pair]^T @ xT   (accumulate over 4 kc)
            def load_w(w_dram, nm):
                tiles = []
                for kc in range(4):
                    wt_t = wpool.tile([P, D], F32R, tag="wv", bufs=4, name=f"w{nm}{kc}")
                    nc.sync.dma_start(
                        out=wt_t[:], in_=w_dram[kc * P : (kc + 1) * P, :]
                    )
                    tiles.append(wt_t)
                return tiles

            PROJS = [
                (wk, kt_sb, N + 1, "k"),
                (wq, qt_sb, 512, "q"),
                (wqs, qst_sb, 512, "qs"),
                (wks, kst_sb, 512, "ks"),
            ]

            def proj_pr(pr, pool, tag, bufs):
                """Emit k/q/qs/ks projection chunks for one head-pair."""
                for w_dram, dst_tiles, ncols, nm in PROJS:
                    wts = []
                    for kc in range(4):
                        wt_t = wpool.tile(
                            [P, P], F32R, tag="w", bufs=8, name=f"w{nm}{pr}{kc}"
                        )
                        nc.sync.dma_start(
                            out=wt_t[:],
                            in_=w_dram[
                                kc * P : (kc + 1) * P, pr * P : (pr + 1) * P
                            ],
                        )
                        wts.append(wt_t)
                    nch = [(s, min(512, ncols - s)) for s in range(0, ncols, 512)]
                    for ci, (jcs, njc) in enumerate(nch):
                        ps = pool.tile(
                            [P, 512], F32, tag=tag, bufs=bufs,
                            name=f"pj{nm}{pr}{ci}",
                        )
                        for kc in range(4):
                            nc.tensor.matmul(
                                out=ps[:, :njc],
                                lhsT=(wts[kc][:]),
                                rhs=(xtf_sb[kc][:, jcs : jcs + njc]),
                                start=(kc == 0),
                                stop=(kc == 3),
                            )
                        nc.vector.tensor_copy(
                            out=dst_tiles[pr][:, jcs : jcs + njc], in_=ps[:, :njc]
                        )

            proj_pr(0, proj_ps, "proj", 3)
            proj_cm.__exit__(None, None, None)
            apool = ctx.enter_context(tc.tile_pool(name="apool", bufs=2))
            etpool = ctx.enter_context(tc.tile_pool(name="etpool", bufs=3))
            big_ps = ctx.enter_context(
                tc.tile_pool(name="big_ps", bufs=1, space="PSUM")
            )
            dt2_ps = ctx.enter_context(
                tc.tile_pool(name="dt2_ps", bufs=1, space="PSUM")
            )
            outT_ps = ctx.enter_context(
                tc.tile_pool(name="outT_ps", bufs=1, space="PSUM")
            )
            self_ps = ctx.enter_context(
                tc.tile_pool(name="self_ps", bufs=2, space="PSUM")
            )
            ones_st = smalls.tile([P, P], F32R, tag="ones", bufs=1)
            nc.gpsimd.dma_start(
                out=ones_st[:],
                in_=bass.AP(
                    tensor=one.ap().tensor, offset=0, ap=[[0, P], [1, P]]
                ),
            )
            # (v projection: streams per-chunk, pipelines with pair 0)

            # v (j-major, ones-augmented): va[j, h, 0:64]=v, va[j, h, 64]=1
            wv_tiles = load_w(wv, "v")

            def vproj_chunk(jc):
                jcs, njc = JB[jc]
                ps = self_ps.tile([P, 512], F32, tag="self", bufs=2, name=f"vp{jc}")
                for kc in range(4):
                    nc.tensor.matmul(
                        out=ps[:njc, :],
                        lhsT=(xtf_sb[kc][:, jcs : jcs + njc]),
                        rhs=(wv_tiles[kc][:]),
                        start=(kc == 0),
                        stop=(kc == 3),
                    )
                psv = ps.rearrange("p (h c) -> p h c", h=8)
                nc.vector.tensor_copy(
                    out=va_v[:njc, jc, :, 0:64], in_=psv[:njc, :, :]
                )
                nc.vector.tensor_copy(
                    out=va_v[:njc, jc, :, 64:65],
                    in_=ones_st[:njc, 0:8].rearrange("p (a b) -> p a b", b=1),
                )

            # ---------------- per-head passes (head pairs) ----------------
            # Pass B (key-major, both heads packed) and pass A (query-major,
            # per head) are independent streams; their chunks are emitted
            # zipped 1:1 so ACT works one stream while PE fills the other.
            for pr in range(4):
                h0, h1 = 2 * pr, 2 * pr + 1

                def passB_chunk(jc, otp2):
                    jcs, njc = JB[jc]
                    if pr == 0:
                        vproj_chunk(jc)
                    dtp2 = dt2_ps.tile(
                        [P, 1024], F32, tag="dotsT", bufs=1, name=f"dt{pr}{jc}"
                    )
                    for half, po in ((0, 0), (1, DH)):
                        nc.tensor.matmul(
                            out=dtp2[:njc, half * 512 : half * 512 + 512],
                            lhsT=(kt_sb[pr][po : po + DH, jcs : jcs + njc]),
                            rhs=(qt_sb[pr][po : po + DH, :]),
                            start=True,
                            stop=True,
                        )
                    if jc < 4:
                        wt, w = WTS[jc], WS[jc]
                        njs = min(njc, R - jcs)
                        for half, po in ((0, 0), (1, DH)):
                            sfp = self_ps.tile(
                                [P, 512], F32, tag="self", bufs=2,
                                name=f"sfb{pr}{jc}{half}",
                            )
                            nc.tensor.matmul(
                                out=sfp[:njs, :w],
                                lhsT=(kst_sb[pr][po : po + DH, jcs : jcs + njs]),
                                rhs=(qst_sb[pr][po : po + DH, wt : wt + w]),
                                start=True,
                                stop=True,
                            )
                            nc.vector.copy_predicated(
                                out=dtp2[:njs, half * 512 + wt : half * 512 + wt + w],
                                mask=mB_sb[jc][:njs, :w],
                                data=sfp[:njs, :w],
                            )
                    et2 = etpool.tile([P, 1024], F32R, tag="et")
                    nc.scalar.activation(
                        out=et2[:njc, :],
                        in_=dtp2[:njc, :],
                        func=mybir.ActivationFunctionType.Exp,
                        scale=SCALE,
                    )
                    for half, h in ((0, h0), (1, h1)):
                        nc.tensor.matmul(
                            out=otp2[:, half * 512 : half * 512 + 512],
                            lhsT=(va_v[:njc, jc, h, :]),
                            rhs=(et2[:njc, half * 512 : half * 512 + 512]),
                            start=(jc == 0),
                            stop=(jc == 15),
                        )

                astate = {}

                def passA_half(i):
                    h, po = (h0, 0) if i < 8 else (h1, DH)
                    t = (i % 8) // 2
                    half = i % 2
                    its, nt = ITS[t]
                    if half == 0:
                        at = apool.tile([P, 2010], F32, tag="attn", name=f"at{pr}{i}")
                        nc.vector.memset(at[:nt, 2009:2010], 1.0)
                        acc = smalls.tile([P, 2], F32, tag="acc", name=f"ac{pr}{i}")
                        astate["at"], astate["acc"] = at, acc
                    at, acc = astate["at"], astate["acc"]
                    chunks = (JA0, JA1)[half]
                    hs = chunks[0][0]
                    hw_ = min(sum(c[1] for c in chunks), N - hs)
                    dp = big_ps.tile(
                        [P, 1024], F32, tag="dots", bufs=1, name=f"dp{pr}{i}"
                    )
                    for jcs, njc in chunks:
                        nc.tensor.matmul(
                            out=dp[:nt, jcs - hs : jcs - hs + njc],
                            lhsT=(qt_sb[pr][po : po + DH, its : its + nt]),
                            rhs=(kt_sb[pr][po : po + DH, jcs : jcs + njc]),
                            start=True,
                            stop=True,
                        )
                    if half == 0:
                        wt, w = WTS[t], WS[t]
                        sfp = self_ps.tile(
                            [P, 512], F32, tag="self", bufs=2, name=f"sfa{pr}{i}"
                        )
                        nc.tensor.matmul(
                            out=sfp[:nt, :w],
                            lhsT=(qst_sb[pr][po : po + DH, its : its + nt]),
                            rhs=(kst_sb[pr][po : po + DH, wt : wt + w]),
                            start=True,
                            stop=True,
                        )
                        nc.vector.copy_predicated(
                            out=dp[:nt, wt : wt + w],
                            mask=mA_sb[t][:nt, :w],
                            data=sfp[:nt, :w],
                        )
                    nc.scalar.activation(
                        out=at[:nt, hs : hs + hw_],
                        in_=dp[:nt, :hw_],
                        func=mybir.ActivationFunctionType.Exp,
                        scale=SCALE,
                        accum_out=acc[:nt, half : half + 1],
                    )
                    if half == 1:
                        ssum = smalls.tile([P, 1], F32, tag="ssum", name=f"ss{pr}{i}")
                        nc.vector.tensor_add(
                            out=ssum[:nt, :], in0=acc[:nt, 0:1], in1=acc[:nt, 1:2]
                        )
                        nc.vector.reciprocal(out=ssum[:nt, :], in_=ssum[:nt, :])
                        nc.vector.tensor_scalar_mul(
                            out=at[:nt, :], in0=at[:nt, :], scalar1=ssum[:nt, :]
                        )
                        nc.sync.dma_start(
                            out=attn_p[h, its : its + nt, :], in_=at[:nt, 0:2009]
                        )

                otp2 = outT_ps.tile(
                    [DH + 1, 1024], F32, tag="outT", bufs=1, name=f"ot2{pr}"
                )
                for i in range(16):
                    passB_chunk(i, otp2)
                    if pr == 3 and i == 4:
                        pass
                    elif pr < 3 and i == 4:
                        proj_pr(pr + 1, self_ps, "self", 2)
                    passA_half(i)

                # free otp2 fast: unnormalized outT + reciprocal of sums
                rsd2 = []
                for half, h in ((0, h0), (1, h1)):
                    nc.vector.tensor_copy(
                        out=ot_sb[h][:], in_=otp2[0:DH, half * 512 : half * 512 + R]
                    )
                    rst = smalls.tile([1, R], F32, tag="rs", bufs=2, name=f"rs{h}")
                    nc.vector.reciprocal(
                        out=rst[:],
                        in_=otp2[DH : DH + 1, half * 512 : half * 512 + R],
                    )
                    rsd = drams.tile([1, R], F32, tag="rsd", bufs=4, name=f"rsd{h}")
                    nc.sync.dma_start(out=rsd[:], in_=rst[:])
                    rsd2.append(rsd)

                # normalize this pair's outT (overlaps with next pair)
                for (half, h), rsd in zip(((0, h0), (1, h1)), rsd2):
                    rb = smalls.tile([DH, R], F32, tag="recb", bufs=2, name=f"rb{h}")
                    nc.gpsimd.dma_start(
                        out=rb[:],
                        in_=bass.AP(
                            tensor=rsd.tensor,
                            offset=rsd.offset,
                            ap=[[0, DH]] + [list(p) for p in rsd.ap[1:]],
                        ),
                    )
                    nc.vector.tensor_mul(
                        out=ot_sb[h][:], in0=ot_sb[h][:], in1=rb[:]
                    )

            # ---------------- output projection (h-outer, 4 accumulators) ----
            ops_t = [
                self_ps.tile([P, 512], F32, tag="self", bufs=2, name="ops0"),
                self_ps.tile([P, 512], F32, tag="self", bufs=2, name="ops1"),
                dt2_ps.tile([P, 512], F32, tag="dotsT", bufs=1, name="ops2"),
                outT_ps.tile([P, 512], F32, tag="outT", bufs=1, name="ops3"),
            ]
            for h in range(H):
                wt_t = wpool.tile([P, D], F32R, tag="wv", bufs=4, name=f"wo{h}")
                nc.sync.dma_start(
                    out=wt_t[:DH, :], in_=wo[h * DH : (h + 1) * DH, :]
                )
                for t, (its, nt) in enumerate(ITS):
                    nc.tensor.matmul(
                        out=ops_t[t][:nt, :],
                        lhsT=(ot_sb[h][:, its : its + nt]),
                        rhs=(wt_t[:DH, :]),
                        start=(h == 0),
                        stop=(h == 7),
                    )
            for t, (its, nt) in enumerate(ITS):
                osb = smalls.tile([P, D], F32, tag="osb", bufs=2, name=f"osb{t}")
                nc.vector.tensor_add(
                    out=osb[:nt, :], in0=ops_t[t][:nt, :], in1=bo_sb[:nt, :]
                )
                nc.sync.dma_start(out=out_p[its : its + nt, :], in_=osb[:nt, :])

    nc.compile()
    return nc


_NC = None


def _get_nc():
    global _NC
    if _NC is None:
        _NC = build_nc()
    return _NC


def make_in_maps(x, w_qkv, w_qk_self, w_out, b_out):
    x = np.asarray(x, np.float32)
    w_qkv = np.asarray(w_qkv, np.float32)
    w_qk_self = np.asarray(w_qk_self, np.float32)
    w_out = np.ascontiguousarray(np.asarray(w_out, np.float32))
    b_out = np.ascontiguousarray(np.asarray(b_out, np.float32))
    wq = np.ascontiguousarray(w_qkv[:, 0:512])
    wk = np.ascontiguousarray(w_qkv[:, 512:1024])
    wv = np.ascontiguousarray(w_qkv[:, 1024:1536])
    wqs = np.ascontiguousarray(w_qk_self[:, 0:512])
    wks = np.ascontiguousarray(w_qk_self[:, 512:1024])
    in_maps = []
    for c in range(8):
        b, blk = c // 4, c % 4
        r0 = R0S[blk]
        xrot = np.roll(x[b], -r0, axis=0)
        xtf = np.zeros((D, N + 1), np.float32)
        xtf[:, :N] = xrot.T
        mAv, mBv = _build_masks(r0)
        in_maps.append(
            dict(
                xtf=xtf, wq=wq, wk=wk, wv=wv, wqs=wqs, wks=wks,
                wo=w_out, bo=b_out, mA=mAv, mB=mBv, one=np.ones(128, np.float32),
            )
        )
    return in_maps


def unshard(results):
    out = np.zeros((2, N, D), np.float32)
    attn = np.zeros((2, H, N, N), np.float32)
    for c in range(8):
        b, blk = c // 4, c % 4
        r0 = R0S[blk]
        nv = min(R, N - r0)
        ap = results[c]["attn_p"]
        attn[b, :, r0 : r0 + nv, :] = np.roll(ap[:, :nv, :], r0, axis=-1)
        out[b, r0 : r0 + nv, :] = results[c]["out_p"][:nv]
    return out, attn


def kernel(x, w_qkv, w_qk_self, w_out, b_out):
    nc = _get_nc()
    in_maps = make_in_maps(x, w_qkv, w_qk_self, w_out, b_out)
    res = run_bass_kernel_spmd(nc, in_maps, core_ids=list(range(8)))
    return unshard(res.results)


# revision 38
# speedup vs baseline: 99.8417x; 1.1295x over previous
"""AgentAwareAttention Trainium2 kernel (8 NeuronCores, SPMD).

Sharding: core c -> batch b=c//4, query-row block r0 = 510*(c%4).
Keys are ROTATED per core by r0 so every SBUF offset is core-independent
(pure SPMD).  Host unshard = np.roll (inverse rotation) + concat.

Per core (all 8 heads, query rows [r0, r0+510), keys all 2009 rotated):
  phase 0: load xT (pre-transposed on host), project kT/ksT/qT/qsT (d-major)
           and v (j-major, ones-augmented 65th column for row sums).
  per head:
    pass B ([key, query] layout): dotsT = k^T q tiles -> blend block-diag
           self scores (copy_predicated w/ host mask) -> exp -> accumulate
           outT[65,510] = v_aug^T @ E^T  (row 64 = softmax denominators)
           -> outT scaled by 1/sums (broadcast DMA) -> SBUF per-head oT.
  pass A ([query, key] layout): dots tiles -> blend -> exp(accum_out=sums)
           -> scale by 1/sums -> DMA attn rows out.
  tail:  out = concat_h(oT)^T @ w_out + b_out  (K=64 accumulating matmuls).
"""

import sys

if "/opt/trn_rl_repo" not in sys.path:
    sys.path.insert(0, "/opt/trn_rl_repo")

import numpy as np

import concourse.bass as bass
import concourse.bacc as bacc
import concourse.tile as tile
from concourse import mybir
from concourse.bass_utils import run_bass_kernel_spmd

F32 = mybir.dt.float32
F32R = mybir.dt.float32r
U8 = mybir.dt.uint8

N = 2009
D = 512
H = 8
DH = 64
SCALE = DH ** -0.5
R = 510            # query rows per core (4 blocks; last block padded)
P = 128
R0S = [0, 510, 1020, 1530]
AGENT_ROWS = 2000

# pass A query i-tiles (start, nrows)
ITS = [(0, 128), (128, 128), (256, 128), (384, 126)]
# self-score window per tile: (col_start, width); windows always inside [0,510)
WTS = [0, 120, 250, 380]
WS = [144, 144, 144, 130]
# pass A key j-chunks (start, ncols) - psum split in two halves
JA0 = [(0, 512), (512, 512)]          # -> half tile 0  [128,1024]
JA1 = [(1024, 512), (1536, 474)]      # -> half tile 1  [128, 985]
# pass B key j-chunks (start, nrows)
JB = [(128 * m, 128) for m in range(15)] + [(1920, 89)]


def _build_masks(r0: int):
    """Block-diagonal blend masks, all indices local/rotated. float32 {0,1}."""
    mA = np.zeros((4, P, 144), np.uint8)
    mB = np.zeros((4, P, 144), np.uint8)
    for t, (its, nt) in enumerate(ITS):
        wt, w = WTS[t], WS[t]
        q = r0 + its + np.arange(nt)              # global query rows
        kl = wt + np.arange(w)                    # local key cols (<510)
        kg = (r0 + kl) % N                        # global key rows
        qa = np.where(q < AGENT_ROWS, q // 10, -1)
        ka = np.where(kg < AGENT_ROWS, kg // 10, -2)
        mA[t, :nt, :w] = (qa[:, None] == ka[None, :]).astype(np.uint8)
    for c in range(4):
        jcs, njc = JB[c]
        wt, w = WTS[c], WS[c]
        jl = jcs + np.arange(njc)                 # local key rows
        kg = (r0 + jl) % N
        kv = (jl < R) & (kg < AGENT_ROWS)
        ka = np.where(kv, kg // 10, -2)
        ql = wt + np.arange(w)                    # local query cols
        qg = r0 + ql
        qv = (ql < R) & (qg < AGENT_ROWS)
        qa = np.where(qv, qg // 10, -1)
        mB[c, :njc, :w] = (ka[:, None] == qa[None, :]).astype(np.uint8)
    return mA, mB




def build_nc(perf_probe=False):
    nc = bacc.Bacc("TRN2", target_bir_lowering=False, debug=False)

    xtf = nc.dram_tensor("xtf", [D, N + 1], F32R, kind="ExternalInput")
    wq = nc.dram_tensor("wq", [D, D], F32R, kind="ExternalInput")
    wk = nc.dram_tensor("wk", [D, D], F32R, kind="ExternalInput")
    wv = nc.dram_tensor("wv", [D, D], F32R, kind="ExternalInput")
    wqs = nc.dram_tensor("wqs", [D, D], F32R, kind="ExternalInput")
    wks = nc.dram_tensor("wks", [D, D], F32R, kind="ExternalInput")
    wo = nc.dram_tensor("wo", [D, D], F32R, kind="ExternalInput")
    bo = nc.dram_tensor("bo", [D], F32, kind="ExternalInput")
    one = nc.dram_tensor("one", [P], F32R, kind="ExternalInput")
    mA = nc.dram_tensor("mA", [4, P, 144], U8, kind="ExternalInput")
    mB = nc.dram_tensor("mB", [4, P, 144], U8, kind="ExternalInput")

    attn_kind = "Internal" if perf_probe else "ExternalOutput"
    attn_p = nc.dram_tensor("attn_p", [H, R, N], F32, kind=attn_kind)
    out_p = nc.dram_tensor("out_p", [R, D], F32, kind="ExternalOutput")

    with tile.TileContext(nc) as tc:
        import contextlib

        ctx = contextlib.ExitStack()
        with ctx:
            persist = ctx.enter_context(tc.tile_pool(name="persist", bufs=1))
            wpool = ctx.enter_context(tc.tile_pool(name="wpool", bufs=4))
            smalls = ctx.enter_context(tc.tile_pool(name="smalls", bufs=4))
            drams = ctx.enter_context(tc.tile_pool(name="drams", bufs=2, space="DRAM"))
            proj_cm = tc.tile_pool(name="proj_ps", bufs=3, space="PSUM")
            proj_ps = proj_cm.__enter__()

            # ---------------- persistent SBUF ----------------
            xtf_sb = [persist.tile([P, N + 1], F32R, tag=f"xtf{kc}", name=f"xtf{kc}") for kc in range(4)]
            kt_sb = [persist.tile([P, N + 1], F32R, tag=f"kt{p}", name=f"kt{p}") for p in range(4)]
            va_sb = persist.tile([P, 16 * 8 * 65], F32R, tag="va")
            va_v = va_sb.rearrange("p (j h c) -> p j h c", j=16, h=8)
            qt_sb = [persist.tile([P, 512], F32R, tag=f"qt{p}", name=f"qt{p}") for p in range(4)]
            qst_sb = [persist.tile([P, 512], F32R, tag=f"qst{p}", name=f"qst{p}") for p in range(4)]
            kst_sb = [persist.tile([P, 512], F32R, tag=f"kst{p}", name=f"kst{p}") for p in range(4)]
            bo_sb = persist.tile([P, D], F32, tag="bo")
            mA_sb = [persist.tile([P, 144], U8, tag=f"mA{t}", name=f"mAt{t}") for t in range(4)]
            mB_sb = [persist.tile([P, 144], U8, tag=f"mB{t}", name=f"mBt{t}") for t in range(4)]
            ot_sb = [persist.tile([DH, R], F32R, tag=f"ot{h}", name=f"ot{h}") for h in range(H)]


            # ---------------- loads ----------------
            xtf_dma_engines = [nc.sync, nc.scalar, nc.gpsimd, nc.gpsimd]
            for kc in range(4):
                xtf_dma_engines[kc].dma_start(
                    out=xtf_sb[kc][:], in_=xtf[kc * P : (kc + 1) * P, :]
                )
            for t in range(4):
                nc.sync.dma_start(out=mA_sb[t][:], in_=mA[t])
                nc.sync.dma_start(out=mB_sb[t][:], in_=mB[t])
            nc.gpsimd.dma_start(
                out=bo_sb[:],
                in_=bass.AP(tensor=bo.ap().tensor, offset=0, ap=[[0, P], [1, D]]),
            )

            # ---------------- projections ----------------
            # kT[pair] [128, N] = wk[:, pair]^T @ xT   (accumulate over 4 kc)
            def load_w(w_dram, nm):
                tiles = []
                for kc in range(4):
                    wt_t = wpool.tile([P, D], F32R, tag="wv", bufs=4, name=f"w{nm}{kc}")
                    nc.sync.dma_start(
                        out=wt_t[:], in_=w_dram[kc * P : (kc + 1) * P, :]
                    )
                    tiles.append(wt_t)
                return tiles

            PROJS = [
                (wk, kt_sb, N + 1, "k"),
                (wq, qt_sb, 512, "q"),
                (wqs, qst_sb, 512, "qs"),
                (wks, kst_sb, 512, "ks"),
            ]

            def proj_pr(pr, pool, tag, bufs):
                """Emit k/q/qs/ks projection chunks for one head-pair."""
                for w_dram, dst_tiles, ncols, nm in PROJS:
                    wts = []
                    for kc in range(4):
                        wt_t = wpool.tile(
                            [P, P], F32R, tag="w", bufs=8, name=f"w{nm}{pr}{kc}"
                        )
                        nc.sync.dma_start(
                            out=wt_t[:],
                            in_=w_dram[
                                kc * P : (kc + 1) * P, pr * P : (pr + 1) * P
                            ],
                        )
                        wts.append(wt_t)
                    nch = [(s, min(512, ncols - s)) for s in range(0, ncols, 512)]
                    for ci, (jcs, njc) in enumerate(nch):
                        ps = pool.tile(
                            [P, 512], F32, tag=tag, bufs=bufs,
                            name=f"pj{nm}{pr}{ci}",
                        )
                        for kc in range(4):
                            nc.tensor.matmul(
                                out=ps[:, :njc],
                                lhsT=(wts[kc][:]),
                                rhs=(xtf_sb[kc][:, jcs : jcs + njc]),
                                start=(kc == 0),
                                stop=(kc == 3),
                            )
                        nc.vector.tensor_copy(
                            out=dst_tiles[pr][:, jcs : jcs + njc], in_=ps[:, :njc]
                        )

            proj_pr(0, proj_ps, "proj", 3)
            proj_cm.__exit__(None, None, None)
            apool = ctx.enter_context(tc.tile_pool(name="apool", bufs=2))
            etpool = ctx.enter_context(tc.tile_pool(name="etpool", bufs=4))
            big_ps = ctx.enter_context(
                tc.tile_pool(name="big_ps", bufs=1, space="PSUM")
            )
            dt2_ps = ctx.enter_context(
                tc.tile_pool(name="dt2_ps", bufs=1, space="PSUM")
            )
            outT_ps = ctx.enter_context(
                tc.tile_pool(name="outT_ps", bufs=1, space="PSUM")
            )
            self_ps = ctx.enter_context(
                tc.tile_pool(name="self_ps", bufs=2, space="PSUM")
            )
            ones_st = smalls.tile([P, P], F32R, tag="ones", bufs=1)
            nc.gpsimd.dma_start(
                out=ones_st[:],
                in_=bass.AP(
                    tensor=one.ap().tensor, offset=0, ap=[[0, P], [1, P]]
                ),
            )
            # (v projection: streams per-chunk, pipelines with pair 0)

            # v (j-major, ones-augmented): va[j, h, 0:64]=v, va[j, h, 64]=1
            wv_tiles = load_w(wv, "v")

            def vproj_chunk(jc):
                jcs, njc = JB[jc]
                ps = self_ps.tile([P, 512], F32, tag="self", bufs=2, name=f"vp{jc}")
                for kc in range(4):
                    nc.tensor.matmul(
                        out=ps[:njc, :],
                        lhsT=(xtf_sb[kc][:, jcs : jcs + njc]),
                        rhs=(wv_tiles[kc][:]),
                        start=(kc == 0),
                        stop=(kc == 3),
                    )
                psv = ps.rearrange("p (h c) -> p h c", h=8)
                nc.vector.tensor_copy(
                    out=va_v[:njc, jc, :, 0:64], in_=psv[:njc, :, :]
                )
                nc.vector.tensor_copy(
                    out=va_v[:njc, jc, :, 64:65],
                    in_=ones_st[:njc, 0:8].rearrange("p (a b) -> p a b", b=1),
                )

            # ---------------- per-head passes (head pairs) ----------------
            # Pass B (key-major, both heads packed) and pass A (query-major,
            # per head) are independent streams; their chunks are emitted
            # zipped 1:1 so ACT works one stream while PE fills the other.
            for pr in range(4):
                h0, h1 = 2 * pr, 2 * pr + 1

                def passB_chunk(jc, otp2):
                    jcs, njc = JB[jc]
                    if pr == 0:
                        vproj_chunk(jc)
                    dtp2 = dt2_ps.tile(
                        [P, 1024], F32, tag="dotsT", bufs=1, name=f"dt{pr}{jc}"
                    )
                    for half, po in ((0, 0), (1, DH)):
                        nc.tensor.matmul(
                            out=dtp2[:njc, half * 512 : half * 512 + 512],
                            lhsT=(kt_sb[pr][po : po + DH, jcs : jcs + njc]),
                            rhs=(qt_sb[pr][po : po + DH, :]),
                            start=True,
                            stop=True,
                        )
                    if jc < 4:
                        wt, w = WTS[jc], WS[jc]
                        njs = min(njc, R - jcs)
                        for half, po in ((0, 0), (1, DH)):
                            sfp = self_ps.tile(
                                [P, 512], F32, tag="self", bufs=2,
                                name=f"sfb{pr}{jc}{half}",
                            )
                            nc.tensor.matmul(
                                out=sfp[:njs, :w],
                                lhsT=(kst_sb[pr][po : po + DH, jcs : jcs + njs]),
                                rhs=(qst_sb[pr][po : po + DH, wt : wt + w]),
                                start=True,
                                stop=True,
                            )
                            nc.vector.copy_predicated(
                                out=dtp2[:njs, half * 512 + wt : half * 512 + wt + w],
                                mask=mB_sb[jc][:njs, :w],
                                data=sfp[:njs, :w],
                            )
                    et2 = etpool.tile([P, 1024], F32R, tag="et")
                    nc.scalar.activation(
                        out=et2[:njc, :],
                        in_=dtp2[:njc, :],
                        func=mybir.ActivationFunctionType.Exp,
                        scale=SCALE,
                    )
                    for half, h in ((0, h0), (1, h1)):
                        nc.tensor.matmul(
                            out=otp2[:, half * 512 : half * 512 + 512],
                            lhsT=(va_v[:njc, jc, h, :]),
                            rhs=(et2[:njc, half * 512 : half * 512 + 512]),
                            start=(jc == 0),
                            stop=(jc == 15),
                        )

                astate = {}

                def passA_half(i):
                    h, po = (h0, 0) if i < 8 else (h1, DH)
                    t = (i % 8) // 2
                    half = i % 2
                    its, nt = ITS[t]
                    if half == 0:
                        at = apool.tile([P, 2010], F32, tag="attn", name=f"at{pr}{i}")
                        nc.vector.memset(at[:nt, 2009:2010], 1.0)
                        acc = smalls.tile([P, 2], F32, tag="acc", name=f"ac{pr}{i}")
                        astate["at"], astate["acc"] = at, acc
                    at, acc = astate["at"], astate["acc"]
                    chunks = (JA0, JA1)[half]
                    hs = chunks[0][0]
                    hw_ = min(sum(c[1] for c in chunks), N - hs)
                    dp = big_ps.tile(
                        [P, 1024], F32, tag="dots", bufs=1, name=f"dp{pr}{i}"
                    )
                    for jcs, njc in chunks:
                        nc.tensor.matmul(
                            out=dp[:nt, jcs - hs : jcs - hs + njc],
                            lhsT=(qt_sb[pr][po : po + DH, its : its + nt]),
                            rhs=(kt_sb[pr][po : po + DH, jcs : jcs + njc]),
                            start=True,
                            stop=True,
                        )
                    if half == 0:
                        wt, w = WTS[t], WS[t]
                        sfp = self_ps.tile(
                            [P, 512], F32, tag="self", bufs=2, name=f"sfa{pr}{i}"
                        )
                        nc.tensor.matmul(
                            out=sfp[:nt, :w],
                            lhsT=(qst_sb[pr][po : po + DH, its : its + nt]),
                            rhs=(kst_sb[pr][po : po + DH, wt : wt + w]),
                            start=True,
                            stop=True,
                        )
                        nc.vector.copy_predicated(
                            out=dp[:nt, wt : wt + w],
                            mask=mA_sb[t][:nt, :w],
                            data=sfp[:nt, :w],
                        )
                    nc.scalar.activation(
                        out=at[:nt, hs : hs + hw_],
                        in_=dp[:nt, :hw_],
                        func=mybir.ActivationFunctionType.Exp,
                        scale=SCALE,
                        accum_out=acc[:nt, half : half + 1],
                    )
                    if half == 1:
                        ssum = smalls.tile([P, 1], F32, tag="ssum", name=f"ss{pr}{i}")
                        nc.vector.tensor_add(
                            out=ssum[:nt, :], in0=acc[:nt, 0:1], in1=acc[:nt, 1:2]
                        )
                        nc.vector.reciprocal(out=ssum[:nt, :], in_=ssum[:nt, :])
                        nc.vector.tensor_scalar_mul(
                            out=at[:nt, :], in0=at[:nt, :], scalar1=ssum[:nt, :]
                        )
                        nc.sync.dma_start(
                            out=attn_p[h, its : its + nt, :], in_=at[:nt, 0:2009]
                        )

                otp2 = outT_ps.tile(
                    [DH + 1, 1024], F32, tag="outT", bufs=1, name=f"ot2{pr}"
                )
                for i in range(16):
                    passB_chunk(i, otp2)
                    if pr == 3 and i == 4:
                        pass
                    elif pr < 3 and i == 4:
                        proj_pr(pr + 1, self_ps, "self", 2)
                    passA_half(i)

                # free otp2 fast: unnormalized outT + reciprocal of sums
                rsd2 = []
                for half, h in ((0, h0), (1, h1)):
                    nc.vector.tensor_copy(
                        out=ot_sb[h][:], in_=otp2[0:DH, half * 512 : half * 512 + R]
                    )
                    rst = smalls.tile([1, R], F32, tag="rs", bufs=2, name=f"rs{h}")
                    nc.vector.reciprocal(
                        out=rst[:],
                        in_=otp2[DH : DH + 1, half * 512 : half * 512 + R],
                    )
                    rsd = drams.tile([1, R], F32, tag="rsd", bufs=4, name=f"rsd{h}")
                    nc.sync.dma_start(out=rsd[:], in_=rst[:])
                    rsd2.append(rsd)

                # normalize this pair's outT (overlaps with next pair)
                for (half, h), rsd in zip(((0, h0), (1, h1)), rsd2):
                    rb = smalls.tile([DH, R], F32, tag="recb", bufs=2, name=f"rb{h}")
                    nc.gpsimd.dma_start(
                        out=rb[:],
                        in_=bass.AP(
                            tensor=rsd.tensor,
                            offset=rsd.offset,
                            ap=[[0, DH]] + [list(p) for p in rsd.ap[1:]],
                        ),
                    )
                    nc.vector.tensor_mul(
                        out=ot_sb[h][:], in0=ot_sb[h][:], in1=rb[:]
                    )

            # ---------------- output projection (h-outer, 4 accumulators) ----
            ops_t = [
                self_ps.tile([P, 512], F32, tag="self", bufs=2, name="ops0"),
                self_ps.tile([P, 512], F32, tag="self", bufs=2, name="ops1"),
                dt2_ps.tile([P, 512], F32, tag="dotsT", bufs=1, name="ops2"),
                outT_ps.tile([P, 512], F32, tag="outT", bufs=1, name="ops3"),
            ]
            for h in range(H):
                wt_t = wpool.tile([P, D], F32R, tag="wv", bufs=4, name=f"wo{h}")
                nc.sync.dma_start(
                    out=wt_t[:DH, :], in_=wo[h * DH : (h + 1) * DH, :]
                )
                for t, (its, nt) in enumerate(ITS):
                    nc.tensor.matmul(
                        out=ops_t[t][:nt, :],
                        lhsT=(ot_sb[h][:, its : its + nt]),
                        rhs=(wt_t[:DH, :]),
                        start=(h == 0),
                        stop=(h == 7),
                    )
            for t, (its, nt) in enumerate(ITS):
                osb = smalls.tile([P, D], F32, tag="osb", bufs=2, name=f"osb{t}")
                nc.vector.tensor_add(
                    out=osb[:nt, :], in0=ops_t[t][:nt, :], in1=bo_sb[:nt, :]
                )
                nc.sync.dma_start(out=out_p[its : its + nt, :], in_=osb[:nt, :])

    nc.compile()
    return nc


_NC = None


def _get_nc():
    global _NC
    if _NC is None:
        _NC = build_nc()
    return _NC


def make_in_maps(x, w_qkv, w_qk_self, w_out, b_out):
    x = np.asarray(x, np.float32)
    w_qkv = np.asarray(w_qkv, np.float32)
    w_qk_self = np.asarray(w_qk_self, np.float32)
    w_out = np.ascontiguousarray(np.asarray(w_out, np.float32))
    b_out = np.ascontiguousarray(np.asarray(b_out, np.float32))
    wq = np.ascontiguousarray(w_qkv[:, 0:512])
    wk = np.ascontiguousarray(w_qkv[:, 512:1024])
    wv = np.ascontiguousarray(w_qkv[:, 1024:1536])
    wqs = np.ascontiguousarray(w_qk_self[:, 0:512])
    wks = np.ascontiguousarray(w_qk_self[:, 512:1024])
    in_maps = []
    for c in range(8):
        b, blk = c // 4, c % 4
        r0 = R0S[blk]
        xrot = np.roll(x[b], -r0, axis=0)
        xtf = np.zeros((D, N + 1), np.float32)
        xtf[:, :N] = xrot.T
        mAv, mBv = _build_masks(r0)
        in_maps.append(
            dict(
                xtf=xtf, wq=wq, wk=wk, wv=wv, wqs=wqs, wks=wks,
                wo=w_out, bo=b_out, mA=mAv, mB=mBv, one=np.ones(128, np.float32),
            )
        )
    return in_maps


def unshard(results):
    out = np.zeros((2, N, D), np.float32)
    attn = np.zeros((2, H, N, N), np.float32)
    for c in range(8):
        b, blk = c // 4, c % 4
        r0 = R0S[blk]
        nv = min(R, N - r0)
        ap = results[c]["attn_p"]
        attn[b, :, r0 : r0 + nv, :] = np.roll(ap[:, :nv, :], r0, axis=-1)
        out[b, r0 : r0 + nv, :] = results[c]["out_p"][:nv]
    return out, attn


def kernel(x, w_qkv, w_qk_self, w_out, b_out):
    nc = _get_nc()
    in_maps = make_in_maps(x, w_qkv, w_qk_self, w_out, b_out)
    res = run_bass_kernel_spmd(nc, in_maps, core_ids=list(range(8)))
    return unshard(res.results)
